# revision 3
# baseline (speedup 1.0000x reference)
"""HebbianConv2d Trainium2 kernel: 3x3 VALID conv with L2-normalized filters +
winner-take-all Hebbian delta_w, data-parallel over batch across 8 NeuronCores.

Self-contained: hardcodes shapes from the problem spec. Each core gets 2 batches.
Per core: conv as 6 shifted matmuls/chunk (3 taps paired via a host-prepared
row-duplicated+shifted x layout -> K=128 contraction), PSUM->SBUF eviction fused
with 1/||w|| scale + bias on ACT, per-chunk row-max on DVE, two-level argmax via
max/max_index, winner row + winner patch fetched with indirect DMA gathers.
"""
import os
import sys

sys.path.insert(0, "/opt/trn_rl_repo")
import numpy as np

import concourse.bass as bass
import concourse.bacc as bacc
import concourse.mybir as mybir
import concourse.tile as tile
from concourse.bass_utils import run_bass_kernel_spmd

B, C_IN, C_OUT, H, W = 16, 64, 128, 128, 128
NCORES = 8
BL = B // NCORES          # batches per core
HO = WO = 126
NPIX = H * W              # 16384
XPAD = 16644              # 16384 + 258 + 2 pad
NCHUNK = 32               # 31 x 512 + 1 x 256 pixel chunks

LAST_EXEC_NS = None
LAST_RESULTS = None
_NC_CACHE = None

f32 = mybir.dt.float32
f32r = mybir.dt.float32r
u32 = mybir.dt.uint32


def _build_nc():
    nc = bacc.Bacc(target_bir_lowering=False, debug=False)

    xd_d = nc.dram_tensor("xd", [BL, 128, XPAD], f32r, kind="ExternalInput")
    xt_d = nc.dram_tensor("xt", [BL, NPIX, C_IN], f32, kind="ExternalInput")
    wp_d = nc.dram_tensor("wp", [128, 6, 128], f32r, kind="ExternalInput")
    wsq_d = nc.dram_tensor("wsq", [128, 576], f32, kind="ExternalInput")
    bias_d = nc.dram_tensor("bias", [128, 1], f32, kind="ExternalInput")
    cob_d = nc.dram_tensor("cob", [128, 1], f32, kind="ExternalInput")

    y_d = nc.dram_tensor("y", [BL, C_OUT, HO, WO], f32, kind="ExternalOutput")
    dw_d = nc.dram_tensor("dw", [C_OUT, 576], f32, kind="ExternalOutput")

    with tile.TileContext(nc) as tc:
        with tc.tile_pool(name="const", bufs=1) as cp, \
             tc.tile_pool(name="xp", bufs=2) as xp, \
             tc.tile_pool(name="yp", bufs=6) as yp, \
             tc.tile_pool(name="sp", bufs=2) as sp, \
             tc.tile_pool(name="pp", bufs=6, space="PSUM") as pp:

            # ---- weight prep: paired stationary + 1/||w|| ----
            wp_sb = cp.tile([128, 6, 128], f32r)
            nc.sync.dma_start(out=wp_sb[:], in_=wp_d[:])
            w_sb = cp.tile([128, 576], f32)
            nc.sync.dma_start(out=w_sb[:], in_=wsq_d[:])
            bias_sb = cp.tile([128, 1], f32)
            nc.sync.dma_start(out=bias_sb[:], in_=bias_d[:])
            cob_sb = cp.tile([128, 1], f32)
            nc.sync.dma_start(out=cob_sb[:], in_=cob_d[:])

            sq = cp.tile([128, 576], f32)
            sumsq = cp.tile([128, 1], f32)
            nc.scalar.activation(out=sq[:], in_=w_sb[:],
                                 func=mybir.ActivationFunctionType.Square,
                                 accum_out=sumsq[:])
            recip = cp.tile([128, 1], f32)
            nc.vector.reciprocal(recip[:], sumsq[:])
            invn = cp.tile([128, 1], f32)
            nc.scalar.sqrt(invn[:], recip[:])

            acc = cp.tile([128, 576], f32)
            xt_flat = xt_d[:].rearrange("b p c -> (b p) c")
            y_rows = y_d[:].rearrange("b c h w -> (b c h) w")

            for b in range(BL):
                xbd = xp.tile([128, XPAD], f32r)
                nsl = 4
                sl = (XPAD + nsl - 1) // nsl
                for s in range(nsl):
                    lo, hi = s * sl, min((s + 1) * sl, XPAD)
                    nc.sync.dma_start(out=xbd[:, lo:hi], in_=xd_d[b, :, lo:hi])

                rmax = sp.tile([128, 126], f32)
                for c in range(NCHUNK):
                    L = 512 if c < 31 else 256
                    nr = 4 if c < 31 else 2
                    p0 = 512 * c
                    ps = pp.tile([128, 512], f32)
                    # kh=0 tap j paired with kh=1 tap j (upper rows hold x shifted +128)
                    for j in range(3):
                        nc.tensor.matmul(out=ps[:, :L], lhsT=wp_sb[:, j, :],
                                         rhs=xbd[:, p0 + j:p0 + j + L],
                                         start=(j == 0), stop=False)
                    # kh=2 taps, K=64 on lower rows
                    for j in range(3):
                        nc.tensor.matmul(out=ps[:, :L], lhsT=wp_sb[0:64, 3 + j, :],
                                         rhs=xbd[0:64, p0 + 256 + j:p0 + 256 + j + L],
                                         start=False, stop=(j == 2))
                    psv = ps[:, :nr * 128].rearrange("p (r w) -> p r w", w=128)[:, :, 0:126]
                    yc = yp.tile([128, 504], f32)
                    nc.scalar.activation(
                        out=yc[:, :nr * 126].rearrange("p (r w) -> p r w", w=126),
                        in_=psv, func=mybir.ActivationFunctionType.Identity,
                        bias=bias_sb[:], scale=invn[:])
                    nc.vector.reduce_max(out=rmax[:, 4 * c:4 * c + nr], in_=psv,
                                         axis=mybir.AxisListType.X)
                    nc.sync.dma_start(out=y_d[b, :, 4 * c:4 * c + nr, :],
                                      in_=yc[:, :nr * 126])

                # ---- argmax level 1: winning row h* ----
                mx8h = sp.tile([128, 8], f32)
                i8h = sp.tile([128, 8], u32)
                nc.vector.max(out=mx8h[:], in_=rmax[:])
                nc.vector.max_index(out=i8h[:], in_max=mx8h[:], in_values=rmax[:])
                hf = sp.tile([128, 1], f32)
                nc.vector.tensor_copy(out=hf[:], in_=i8h[:, 0:1])
                rowf = sp.tile([128, 1], f32)
                nc.vector.tensor_tensor(out=rowf[:], in0=cob_sb[:], in1=hf[:],
                                        op=mybir.AluOpType.add)
                rowu = sp.tile([128, 1], u32)
                nc.vector.tensor_copy(out=rowu[:], in_=rowf[:])

                # ---- fetch winning y row, argmax level 2: w* ----
                ywin = sp.tile([128, 126], f32)
                nc.gpsimd.indirect_dma_start(
                    out=ywin[:], out_offset=None, in_=y_rows,
                    in_offset=bass.IndirectOffsetOnAxis(ap=rowu[:], axis=0),
                    element_offset=b * C_OUT * HO * WO)
                mx8w = sp.tile([128, 8], f32)
                i8w = sp.tile([128, 8], u32)
                nc.vector.max(out=mx8w[:], in_=ywin[:])
                nc.vector.max_index(out=i8w[:], in_max=mx8w[:], in_values=ywin[:])
                wfl = sp.tile([128, 1], f32)
                nc.vector.tensor_copy(out=wfl[:], in_=i8w[:, 0:1])

                # pixel index p* = h**128 + w*
                pixf = sp.tile([128, 1], f32)
                nc.vector.tensor_scalar(out=pixf[:], in0=hf[:], scalar1=128.0,
                                        scalar2=None, op0=mybir.AluOpType.mult)
                pixf2 = sp.tile([128, 1], f32)
                nc.vector.tensor_tensor(out=pixf2[:], in0=pixf[:], in1=wfl[:],
                                        op=mybir.AluOpType.add)
                pixu = sp.tile([128, 1], u32)
                nc.vector.tensor_copy(out=pixu[:], in_=pixf2[:])

                # ---- gather winning patch rows from channel-last x ----
                gat = sp.tile([128, 576], f32)
                for kh in range(3):
                    nc.gpsimd.indirect_dma_start(
                        out=gat[:, kh * 192:(kh + 1) * 192], out_offset=None,
                        in_=xt_flat,
                        in_offset=bass.IndirectOffsetOnAxis(ap=pixu[:], axis=0),
                        element_offset=(b * NPIX + kh * W) * C_IN)
                gperm = gat[:].rearrange("p (kh kw ci) -> p ci (kh kw)",
                                         kh=3, kw=3, ci=64)
                if b == 0:
                    nc.vector.tensor_copy(out=acc[:], in_=gperm)
                else:
                    nc.vector.tensor_tensor(out=acc[:], in0=acc[:], in1=gperm,
                                            op=mybir.AluOpType.add)

            dwout = cp.tile([128, 576], f32)
            nc.scalar.mul(dwout[:], acc[:], 1.0 / B)
            nc.sync.dma_start(out=dw_d[:], in_=dwout[:])

    nc.compile()
    return nc


def kernel(x, weight, bias):
    global LAST_EXEC_NS, LAST_RESULTS, _NC_CACHE
    x = np.ascontiguousarray(x, dtype=np.float32)
    weight = np.ascontiguousarray(weight, dtype=np.float32)
    bias = np.ascontiguousarray(bias, dtype=np.float32)

    if _NC_CACHE is None:
        _NC_CACHE = _build_nc()
    nc = _NC_CACHE

    # paired stationary weights: wp[ci, j, co] = w[co, ci, 0, j]; upper ci+64 -> kh=1
    wp = np.zeros((128, 6, 128), np.float32)
    for j in range(3):
        wp[0:64, j, :] = weight[:, :, 0, j].T
        wp[64:128, j, :] = weight[:, :, 1, j].T
        wp[0:64, 3 + j, :] = weight[:, :, 2, j].T
    wsq = weight.reshape(128, 576)
    bias_in = bias.reshape(128, 1)
    cob = (np.arange(128, dtype=np.float32) * HO).reshape(128, 1)

    in_maps = []
    for core in range(NCORES):
        xs = x[core * BL:(core + 1) * BL]                      # (BL, 64, 128, 128)
        flat = xs.reshape(BL, C_IN, NPIX)
        xd = np.zeros((BL, 128, XPAD), np.float32)
        xd[:, 0:64, 0:NPIX] = flat
        xd[:, 64:128, 0:NPIX - W] = flat[:, :, W:]
        xt = np.ascontiguousarray(xs.transpose(0, 2, 3, 1)).reshape(BL, NPIX, C_IN)
        in_maps.append({"xd": xd, "xt": xt, "wp": wp, "wsq": wsq,
                        "bias": bias_in, "cob": cob})

    trace = bool(int(os.environ.get("BASS_KERNEL_TRACE", "0")))
    if trace:
        try:
            import antenv.axon_hooks  # noqa: F401
        except ImportError:
            trace = False
    try:
        res = run_bass_kernel_spmd(nc, in_maps, list(range(NCORES)), trace=trace)
    except Exception:
        if not trace:
            raise
        res = run_bass_kernel_spmd(nc, in_maps, list(range(NCORES)), trace=False)
    LAST_EXEC_NS = res.exec_time_ns
    LAST_RESULTS = res

    y = np.concatenate([res.results[i]["y"] for i in range(NCORES)], axis=0)
    dw = np.zeros((128, 576), np.float64)
    for i in range(NCORES):
        dw += res.results[i]["dw"]
    delta_w = dw.astype(np.float32).reshape(C_OUT, C_IN, 3, 3)
    return y, delta_w


# revision 7
# speedup vs baseline: 1.0071x; 1.0071x over previous
"""HebbianConv2d Trainium2 kernel: 3x3 VALID conv with L2-normalized filters +
winner-take-all Hebbian delta_w, data-parallel over batch across 8 NeuronCores.

Self-contained: hardcodes shapes from the problem spec. Each core gets 2 batches.
Per core: conv as 6 shifted matmuls/chunk (3 taps paired via a host-prepared
row-duplicated+shifted x layout -> K=128 contraction), PSUM->SBUF eviction fused
with 1/||w|| scale + bias on ACT, per-chunk row-max on DVE, two-level argmax via
max/max_index, winner row + winner patch fetched with indirect DMA gathers.
"""
import os
import sys

sys.path.insert(0, "/opt/trn_rl_repo")
import numpy as np

import concourse.bass as bass
import concourse.bacc as bacc
import concourse.mybir as mybir
import concourse.tile as tile
from concourse.bass_utils import run_bass_kernel_spmd

B, C_IN, C_OUT, H, W = 16, 64, 128, 128, 128
NCORES = 8
BL = B // NCORES          # batches per core
HO = WO = 126
NPIX = H * W              # 16384
XPAD = 16644              # 16384 + 258 + 2 pad
NCHUNK = 32               # 31 x 512 + 1 x 256 pixel chunks

LAST_EXEC_NS = None
LAST_RESULTS = None
_NC_CACHE = None

f32 = mybir.dt.float32
f32r = mybir.dt.float32r
u32 = mybir.dt.uint32


def _build_nc():
    nc = bacc.Bacc(target_bir_lowering=False, debug=False)

    xd_d = nc.dram_tensor("xd", [BL, 128, XPAD], f32r, kind="ExternalInput")
    xt_d = nc.dram_tensor("xt", [BL, NPIX, C_IN], f32, kind="ExternalInput")
    wp_d = nc.dram_tensor("wp", [128, 6, 128], f32r, kind="ExternalInput")
    wsq_d = nc.dram_tensor("wsq", [128, 576], f32, kind="ExternalInput")
    bias_d = nc.dram_tensor("bias", [128, 1], f32, kind="ExternalInput")
    cob_d = nc.dram_tensor("cob", [128, 1], f32, kind="ExternalInput")

    y_d = nc.dram_tensor("y", [BL, C_OUT, HO, WO], f32, kind="ExternalOutput")
    dw_d = nc.dram_tensor("dw", [C_OUT, 576], f32, kind="ExternalOutput")

    with tile.TileContext(nc) as tc:
        with tc.tile_pool(name="const", bufs=1) as cp, \
             tc.tile_pool(name="xp", bufs=2) as xp, \
             tc.tile_pool(name="yp", bufs=6) as yp, \
             tc.tile_pool(name="sp", bufs=2) as sp, \
             tc.tile_pool(name="pp", bufs=3, space="PSUM") as pp:

            # ---- first x slice + stationary weights first so PE starts ASAP ----
            xbd0 = xp.tile([128, XPAD], f32r, tag="xbd")
            nc.sync.dma_start(out=xbd0[:, 0:1536], in_=xd_d[0, :, 0:1536])
            wp_sb = cp.tile([128, 6, 128], f32r)
            nc.sync.dma_start(out=wp_sb[:], in_=wp_d[:])
            w_sb = cp.tile([128, 576], f32)
            nc.sync.dma_start(out=w_sb[:], in_=wsq_d[:])
            bias_sb = cp.tile([128, 1], f32)
            nc.sync.dma_start(out=bias_sb[:], in_=bias_d[:])
            cob_sb = cp.tile([128, 1], f32)
            nc.sync.dma_start(out=cob_sb[:], in_=cob_d[:])

            sq = cp.tile([128, 576], f32)
            sumsq = cp.tile([128, 1], f32)
            nc.scalar.activation(out=sq[:], in_=w_sb[:],
                                 func=mybir.ActivationFunctionType.Square,
                                 accum_out=sumsq[:])
            recip = cp.tile([128, 1], f32)
            nc.vector.reciprocal(recip[:], sumsq[:])
            invn = cp.tile([128, 1], f32)
            nc.scalar.sqrt(invn[:], recip[:])

            acc = cp.tile([128, 576], f32)
            xt_flat = xt_d[:].rearrange("b p c -> (b p) c")
            y_rows = y_d[:].rearrange("b c h w -> (b c h) w")

            for b in range(BL):
                if b == 0:
                    xbd = xbd0
                    bounds = [1536, 6656, 11776, XPAD]
                else:
                    xbd = xp.tile([128, XPAD], f32r, tag="xbd")
                    bounds = [0, 5548, 11096, XPAD]
                for lo, hi in zip(bounds[:-1], bounds[1:]):
                    nc.sync.dma_start(out=xbd[:, lo:hi], in_=xd_d[b, :, lo:hi])

                rmax = sp.tile([128, 126], f32)
                for g in range(NCHUNK // 2):
                    ps = pp.tile([128, 1024], f32)
                    nr_tot = 8 if g < 15 else 6
                    for sub in range(2):
                        c = 2 * g + sub
                        L = 512 if c < 31 else 256
                        p0 = 512 * c
                        q0 = 512 * sub
                        # kh=0 tap j paired with kh=1 tap j (upper rows: x shifted +128)
                        for j in range(3):
                            nc.tensor.matmul(out=ps[:, q0:q0 + L], lhsT=wp_sb[:, j, :],
                                             rhs=xbd[:, p0 + j:p0 + j + L],
                                             start=(j == 0), stop=False)
                        # kh=2 taps, K=64 on lower rows
                        for j in range(3):
                            nc.tensor.matmul(out=ps[:, q0:q0 + L],
                                             lhsT=wp_sb[0:64, 3 + j, :],
                                             rhs=xbd[0:64, p0 + 256 + j:p0 + 256 + j + L],
                                             start=False, stop=(j == 2))
                    psv = ps[:, :nr_tot * 128].rearrange("p (r w) -> p r w", w=128)[:, :, 0:126]
                    yc = yp.tile([128, 1008], f32)
                    nc.scalar.activation(
                        out=yc[:, :nr_tot * 126].rearrange("p (r w) -> p r w", w=126),
                        in_=psv, func=mybir.ActivationFunctionType.Identity,
                        bias=bias_sb[:], scale=invn[:])
                    nc.vector.reduce_max(out=rmax[:, 8 * g:8 * g + nr_tot], in_=psv,
                                         axis=mybir.AxisListType.X)
                    nc.sync.dma_start(out=y_d[b, :, 8 * g:8 * g + nr_tot, :],
                                      in_=yc[:, :nr_tot * 126])

                # ---- argmax level 1: winning row h* ----
                mx8h = sp.tile([128, 8], f32)
                i8h = sp.tile([128, 8], u32)
                nc.vector.max(out=mx8h[:], in_=rmax[:])
                nc.vector.max_index(out=i8h[:], in_max=mx8h[:], in_values=rmax[:])
                hf = sp.tile([128, 1], f32)
                nc.vector.tensor_copy(out=hf[:], in_=i8h[:, 0:1])
                rowf = sp.tile([128, 1], f32)
                nc.vector.tensor_tensor(out=rowf[:], in0=cob_sb[:], in1=hf[:],
                                        op=mybir.AluOpType.add)
                rowu = sp.tile([128, 1], u32)
                nc.vector.tensor_copy(out=rowu[:], in_=rowf[:])

                # ---- fetch winning y row, argmax level 2: w* ----
                ywin = sp.tile([128, 126], f32)
                nc.gpsimd.indirect_dma_start(
                    out=ywin[:], out_offset=None, in_=y_rows,
                    in_offset=bass.IndirectOffsetOnAxis(ap=rowu[:], axis=0),
                    element_offset=b * C_OUT * HO * WO)
                mx8w = sp.tile([128, 8], f32)
                i8w = sp.tile([128, 8], u32)
                nc.vector.max(out=mx8w[:], in_=ywin[:])
                nc.vector.max_index(out=i8w[:], in_max=mx8w[:], in_values=ywin[:])
                wfl = sp.tile([128, 1], f32)
                nc.vector.tensor_copy(out=wfl[:], in_=i8w[:, 0:1])

                # pixel index p* = h**128 + w*
                pixf = sp.tile([128, 1], f32)
                nc.vector.tensor_scalar(out=pixf[:], in0=hf[:], scalar1=128.0,
                                        scalar2=None, op0=mybir.AluOpType.mult)
                pixf2 = sp.tile([128, 1], f32)
                nc.vector.tensor_tensor(out=pixf2[:], in0=pixf[:], in1=wfl[:],
                                        op=mybir.AluOpType.add)
                pixu = sp.tile([128, 1], u32)
                nc.vector.tensor_copy(out=pixu[:], in_=pixf2[:])

                # ---- gather winning patch rows from channel-last x ----
                gat = sp.tile([128, 576], f32)
                for kh in range(3):
                    nc.gpsimd.indirect_dma_start(
                        out=gat[:, kh * 192:(kh + 1) * 192], out_offset=None,
                        in_=xt_flat,
                        in_offset=bass.IndirectOffsetOnAxis(ap=pixu[:], axis=0),
                        element_offset=(b * NPIX + kh * W) * C_IN)
                gperm = gat[:].rearrange("p (kh kw ci) -> p ci (kh kw)",
                                         kh=3, kw=3, ci=64)
                if b == 0:
                    nc.vector.tensor_copy(out=acc[:], in_=gperm)
                else:
                    nc.vector.tensor_tensor(out=acc[:], in0=acc[:], in1=gperm,
                                            op=mybir.AluOpType.add)

            dwout = cp.tile([128, 576], f32)
            nc.scalar.mul(dwout[:], acc[:], 1.0 / B)
            nc.sync.dma_start(out=dw_d[:], in_=dwout[:])

    nc.compile()
    return nc


def kernel(x, weight, bias):
    global LAST_EXEC_NS, LAST_RESULTS, _NC_CACHE
    x = np.ascontiguousarray(x, dtype=np.float32)
    weight = np.ascontiguousarray(weight, dtype=np.float32)
    bias = np.ascontiguousarray(bias, dtype=np.float32)

    if _NC_CACHE is None:
        _NC_CACHE = _build_nc()
    nc = _NC_CACHE

    # paired stationary weights: wp[ci, j, co] = w[co, ci, 0, j]; upper ci+64 -> kh=1
    wp = np.zeros((128, 6, 128), np.float32)
    for j in range(3):
        wp[0:64, j, :] = weight[:, :, 0, j].T
        wp[64:128, j, :] = weight[:, :, 1, j].T
        wp[0:64, 3 + j, :] = weight[:, :, 2, j].T
    wsq = weight.reshape(128, 576)
    bias_in = bias.reshape(128, 1)
    cob = (np.arange(128, dtype=np.float32) * HO).reshape(128, 1)

    in_maps = []
    for core in range(NCORES):
        xs = x[core * BL:(core + 1) * BL]                      # (BL, 64, 128, 128)
        flat = xs.reshape(BL, C_IN, NPIX)
        xd = np.zeros((BL, 128, XPAD), np.float32)
        xd[:, 0:64, 0:NPIX] = flat
        xd[:, 64:128, 0:NPIX - W] = flat[:, :, W:]
        xt = np.ascontiguousarray(xs.transpose(0, 2, 3, 1)).reshape(BL, NPIX, C_IN)
        in_maps.append({"xd": xd, "xt": xt, "wp": wp, "wsq": wsq,
                        "bias": bias_in, "cob": cob})

    trace = bool(int(os.environ.get("BASS_KERNEL_TRACE", "0")))
    if trace:
        try:
            import antenv.axon_hooks  # noqa: F401
        except ImportError:
            trace = False
    try:
        res = run_bass_kernel_spmd(nc, in_maps, list(range(NCORES)), trace=trace)
    except Exception:
        if not trace:
            raise
        res = run_bass_kernel_spmd(nc, in_maps, list(range(NCORES)), trace=False)
    LAST_EXEC_NS = res.exec_time_ns
    LAST_RESULTS = res

    y = np.concatenate([res.results[i]["y"] for i in range(NCORES)], axis=0)
    dw = np.zeros((128, 576), np.float64)
    for i in range(NCORES):
        dw += res.results[i]["dw"]
    delta_w = dw.astype(np.float32).reshape(C_OUT, C_IN, 3, 3)
    return y, delta_w
